# revision 1
# baseline (speedup 1.0000x reference)
import numpy as np
import jax
import jax.numpy as jnp

# nn_AttnFFN: Attention4D token mixer + conv-MLP, B=64, dim=384, res=16.
# Sharding: data-parallel over batch across the available NeuronCores
# (B=64 -> 8 per core on 8 cores), weights replicated.

_HEADS = 8
_KD = 32
_D = 128
_RES = 16
_N = _RES * _RES


def _c1(x, w, b):
    # 1x1 conv as channel GEMM: w [O,C], x [B,C,H,W]
    return jnp.einsum('oc,bchw->bohw', w, x) + b[None, :, None, None]


def _dw3(x, w, b):
    y = jax.lax.conv_general_dilated(
        x, w, (1, 1), 'SAME',
        dimension_numbers=('NCHW', 'OIHW', 'NCHW'),
        feature_group_count=x.shape[1])
    return y + b[None, :, None, None]


def _bn(x, g, b, m, v):
    s = (g * jax.lax.rsqrt(v + 1e-5))
    return (x - m[None, :, None, None]) * s[None, :, None, None] + b[None, :, None, None]


def _forward(x, qw, qb, kw, kb, vw, vb, vlw, vlb, th1w, th1b, th2w, th2b, ab,
             pw, pb, f1w, f1b, g1, b1, m1, v1, mw, mb, gm, bm, mm, vm,
             f2w, f2b, g2, b2, m2, v2, bias_idxs):
    B, C, H, W = x.shape
    heads = _HEADS
    kd = _KD
    d = _D
    N = H * W
    scale = kd ** -0.5
    q = _c1(x, qw, qb).reshape(B, heads, kd, N).transpose(0, 1, 3, 2)
    k = _c1(x, kw, kb).reshape(B, heads, kd, N)
    v4 = _c1(x, vw, vb)
    v_local = _dw3(v4, vlw, vlb)
    v = v4.reshape(B, heads, d, N).transpose(0, 1, 3, 2)
    bias = ab[:, bias_idxs]
    attn = jnp.einsum('bhnk,bhkm->bhnm', q, k) * scale + bias[None]
    attn = jnp.einsum('og,bgnm->bonm', th1w, attn) + th1b[None, :, None, None]
    attn = jax.nn.softmax(attn, axis=-1)
    attn = jnp.einsum('og,bgnm->bonm', th2w, attn) + th2b[None, :, None, None]
    o = jnp.einsum('bhnm,bhmd->bhnd', attn, v)
    o = o.transpose(0, 1, 3, 2).reshape(B, heads * d, H, W) + v_local
    o = _c1(jax.nn.relu(o), pw, pb)
    x = x + o
    h = jax.nn.relu(_bn(_c1(x, f1w, f1b), g1, b1, m1, v1))
    h = jax.nn.relu(_bn(_dw3(h, mw, mb), gm, bm, mm, vm))
    h = _bn(_c1(h, f2w, f2b), g2, b2, m2, v2)
    return x + h


_ARG_NAMES = ['qw', 'qb', 'kw', 'kb', 'vw', 'vb', 'vlw', 'vlb', 'th1w', 'th1b',
              'th2w', 'th2b', 'ab', 'pw', 'pb', 'f1w', 'f1b', 'g1', 'b1', 'm1',
              'v1', 'mw', 'mb', 'gm', 'bm', 'mm', 'vm', 'f2w', 'f2b', 'g2',
              'b2', 'm2', 'v2', 'bias_idxs']

_pmapped = None


def _get_pmapped(n_dev):
    global _pmapped
    if _pmapped is None:
        _pmapped = jax.pmap(_forward, in_axes=(0,) + (None,) * len(_ARG_NAMES))
    return _pmapped


def kernel(**inputs):
    x = np.asarray(inputs['x'])
    args = [jnp.asarray(inputs[n]) for n in _ARG_NAMES]
    B = x.shape[0]
    devs = jax.devices()
    n_dev = min(len(devs), B)
    # largest divisor of B that is <= n_dev
    while B % n_dev != 0:
        n_dev -= 1
    if n_dev > 1:
        xs = jnp.asarray(x.reshape((n_dev, B // n_dev) + x.shape[1:]))
        fn = _get_pmapped(n_dev)
        out = fn(xs, *args)
        out = np.asarray(out).reshape((B,) + out.shape[2:])
    else:
        out = np.asarray(jax.jit(_forward)(jnp.asarray(x), *args))
    return out.astype(np.float32)
